# revision 5
# baseline (speedup 1.0000x reference)
"""DeepAR (2-layer LSTM, B=1024, W=288, H=128) forward on 8 Trainium2 cores.

Pure data-parallel: batch 1024 -> 128 per core; weights replicated.

Device layout is "transposed activations": every on-chip tensor is
(feature_dim = partitions, batch = free).  Cell math per step, with the
tanh identity sigmoid(x) = (tanh(x/2)+1)/2 and states C = 2c, H = 2h
(weights consuming h are pre-halved; i/f gate rows pre-halved, o rows
pre-halved for the sigmoid's scale=2, g rows full):

    psum A = [g|f|i] gate blocks (one bank), psum B = [o] (another bank)
    t_ext[:,128:512] = tanh(A)                  (ACT, 384 wide)
    S              = sigmoid(2 * B) = (To+1)/2  (ACT, 128 wide, off-chain)
    [v|u]          = (t_ext[:,256:512] + 1) * t_ext[:,0:256]   (DVE stt)
    C_new          = 0.5*v + u  -> next t_ext[:,0:128]         (DVE stt)
    H_new          = S * C_new * poly(C_new^2)                 (custom DVE op)

where poly is a degree-5 odd minimax fit of 2*tanh(0.5*X) on |X|<=2.2
(|C| stays under ~1.75 for this model; checked against the reference).
The custom DVE op fuses tanh(c) and the output-gate multiply into one
vector instruction, so each cell touches the scalar engine only once on
the critical path.

t_ext layout (bf16, 512 cols): [C_prev | Tg | Tf | Ti]; the C slot of the
NEXT step's tile is written by this step's c-op, which makes the uv
operand [C|Tg] a single contiguous access pattern.

Prediction-phase feedback (prev_y = mean_{t-1}) is folded into the
recurrence as a rank-1 matrix Wfb = Wi0[:,0] (x) (0.5*meanW) applied to
H2; means are computed on the host from the exported H2 states.
"""

import ml_dtypes
import numpy as np

BF16 = ml_dtypes.bfloat16

B = 1024
SEQ, PRED = 192, 96
W = SEQ + PRED  # 288
HID = 128
NCORES = 8
BS = B // NCORES  # 128
IN = 67
KX = IN + 2  # + ones row (bias1) + indicator row (pred feedback bias)
G4 = 4 * HID  # 512
# torch gate order (i, f, g, o) -> device order (g, f, i, o)
GATE_PERM = [2, 1, 0, 3]
X_CHUNK = 16  # scan steps per input-DMA chunk
WOFF = {"wi0": 0, "wh0": 512, "wi1": 1024, "wh1": 1536, "wfb": 2048,
        "b2m": 2560, "bones": 2688}
WCOLS = 2688 + 512  # 3200

# degree-5 odd minimax of 2*tanh(0.5*X) ~= X*(P0 + P1*X^2 + P2*X^4), |X|<=2.2
P0, P1, P2 = 0.99558505, -0.07501307, 0.0040895


def _perm_rows(w):
    """(4H, X) or (4H,) -> gate-permuted; f/i/o rows halved (tanh trick)."""
    w = w.reshape(4, HID, -1) if w.ndim == 2 else w.reshape(4, HID, 1)
    w = w[GATE_PERM].astype(np.float64).copy()  # (g, f, i, o)
    w[1] *= 0.5  # f
    w[2] *= 0.5  # i
    w[3] *= 0.5  # o
    return w  # (4, HID, X)


def _as_blocksT(w4):
    """(4, HID, K) -> (K, 4*HID) with gate blocks along columns (lhsT form)."""
    k = w4.shape[2]
    out = np.zeros((k, G4), np.float64)
    for g in range(4):
        out[:, g * HID:(g + 1) * HID] = w4[g].T
    return out


def host_prep(inputs):
    """All data-movement-only preprocessing + weight folding. Returns dict."""
    f32 = np.float32
    ge = np.asarray(inputs["given_enc"], f32)
    x_enc = np.asarray(inputs["x_enc"], f32)
    xm = np.asarray(inputs["x_mark_enc"], f32)
    mx = np.asarray(inputs["meta_x"], f32)
    tembs = [np.asarray(inputs[f"time_emb{i}"], f32) for i in range(3)]
    membs = [np.asarray(inputs[f"meta_emb{i}"], f32) for i in range(2)]

    tcat = ge[:, :, 4:7].astype(np.int32)
    time_feat = np.concatenate(
        [ge[:, :, :4]] + [tembs[i][tcat[:, :, i]] for i in range(3)], axis=-1
    )  # (B, W, 28)
    mcat = mx[:, 2:4].astype(np.int32)
    meta_feat = np.concatenate(
        [mx[:, :2]] + [membs[i][mcat[:, i]] for i in range(2)], axis=-1
    )  # (B, 34)

    nm = x_enc.mean(axis=1, keepdims=True)  # (B,1,1)
    xc = x_enc - nm
    ns = np.sqrt(xc.var(axis=1, keepdims=True) + 1e-5)
    xn = (xc / ns).astype(f32)  # (B, SEQ, 1)

    teacher = np.zeros((B, W, 1), f32)
    teacher[:, 0] = xn[:, 0]
    teacher[:, 1:SEQ] = xn[:, : SEQ - 1]
    ones = np.ones((B, W, 1), f32)
    ind = np.zeros((B, W, 1), f32)
    ind[:, SEQ:] = 1.0
    xfeat = np.concatenate(
        [teacher, time_feat, xm,
         np.broadcast_to(meta_feat[:, None, :], (B, W, 34)), ones, ind],
        axis=-1,
    )  # (B, W, 69)

    Wi0 = np.asarray(inputs["W_ih0"], np.float64)  # (512, 67)
    Wh0 = np.asarray(inputs["W_hh0"], np.float64)
    Wi1 = np.asarray(inputs["W_ih1"], np.float64)
    Wh1 = np.asarray(inputs["W_hh1"], np.float64)
    b1 = np.asarray(inputs["b_ih0"], np.float64) + np.asarray(inputs["b_hh0"], np.float64)
    b2 = np.asarray(inputs["b_ih1"], np.float64) + np.asarray(inputs["b_hh1"], np.float64)
    meanW = np.asarray(inputs["mean_W"], np.float64)  # (1, 128)
    mean_b = float(np.asarray(inputs["mean_b"]).reshape(()))

    wfb_full = Wi0[:, 0:1] @ (0.5 * meanW)  # consumes H2 = 2*h2
    bias_fb = Wi0[:, 0] * mean_b  # (512,)

    wi0T = _as_blocksT(_perm_rows(Wi0))  # (67, 512)
    wi0T_aug = np.zeros((KX, G4), np.float64)
    wi0T_aug[:IN] = wi0T
    wi0T_aug[IN] = _as_blocksT(_perm_rows(b1)).reshape(G4)  # ones row: bias1
    wi0T_aug[IN + 1] = _as_blocksT(_perm_rows(bias_fb)).reshape(G4)  # indicator
    wh0T = _as_blocksT(_perm_rows(Wh0) * 0.5)  # *0.5: h state is H = 2h
    wi1T = _as_blocksT(_perm_rows(Wi1) * 0.5)
    wh1T = _as_blocksT(_perm_rows(Wh1) * 0.5)
    wfbT = _as_blocksT(_perm_rows(wfb_full))  # (128, 512)

    b2m = _perm_rows(b2).reshape(4, HID)
    bones = np.zeros((4, G4), f32)
    for g in range(4):
        bones[g, g * HID:(g + 1) * HID] = 1.0

    # per-core transposed inputs: (KX, W*BS), feature on partitions
    xt_cores = []
    for c in range(NCORES):
        xf = xfeat[c * BS:(c + 1) * BS]  # (BS, W, KX)
        xt = np.ascontiguousarray(xf.transpose(2, 1, 0)).reshape(KX, W * BS)
        xt_cores.append(xt.astype(BF16))

    # Pack every weight into one (128, WCOLS) tensor -> single DMA.
    wconst = np.zeros((HID, WCOLS), BF16)
    wconst[:KX, WOFF["wi0"]:WOFF["wi0"] + G4] = wi0T_aug
    wconst[:, WOFF["wh0"]:WOFF["wh0"] + G4] = wh0T
    wconst[:, WOFF["wi1"]:WOFF["wi1"] + G4] = wi1T
    wconst[:, WOFF["wh1"]:WOFF["wh1"] + G4] = wh1T
    wconst[:, WOFF["wfb"]:WOFF["wfb"] + G4] = wfbT
    wconst[:4, WOFF["b2m"]:WOFF["b2m"] + HID] = b2m
    wconst[:4, WOFF["bones"]:WOFF["bones"] + G4] = bones

    return dict(
        xt_cores=xt_cores,
        wconst=wconst,
        weights=dict(
            wi0=wi0T_aug.astype(f32), wh0=wh0T.astype(f32),
            wi1=wi1T.astype(f32), wh1=wh1T.astype(f32),
            wfb=wfbT.astype(f32), b2m=b2m.astype(f32), bones=bones,
        ),
        meanW_h=(0.5 * meanW).astype(f32), mean_b=mean_b,
        norm_std=ns.astype(f32), norm_mean=nm.astype(f32),
    )


def host_post(h2_cores, prep):
    """h2_cores: list of (PRED, HID, BS) arrays of H2=2*h2. -> (B, PRED, 1)."""
    meanW_h = prep["meanW_h"][0]  # (HID,)
    out = np.empty((B, PRED, 1), np.float32)
    for c, h2 in enumerate(h2_cores):
        mn = np.einsum("h,thb->bt", meanW_h, h2.astype(np.float32)) + prep["mean_b"]
        out[c * BS:(c + 1) * BS, :, 0] = mn
    out = out * prep["norm_std"] + prep["norm_mean"]
    return out.astype(np.float32)


_TANH5_MUL = None


def _get_tanh5_mul():
    """Register the fused (sigmoid-gate * poly-tanh) custom DVE op.

    out = in0 * (s0 + t*(s1 + t*imm2)) * in1, t = in0^2.  Registered via the
    documented dve_ops extension point (OPS list + derived tables)."""
    global _TANH5_MUL
    if _TANH5_MUL is not None:
        return _TANH5_MUL
    import concourse.dve_ops as dve_ops
    from concourse.dve_ops import DveOp, get_dve_sub_opcode
    from concourse.dve_spec import Spec, Src0, Src1, C0, C1, C2, sq, lower
    from concourse.dve_uop import DveOpSpec

    name = "TANH5_MUL"
    for existing in dve_ops.OPS:
        if existing.name == name:
            _TANH5_MUL = existing
            return existing
    t = sq(Src0)
    body = (Src0 * (C0 + t * (C1 + t * C2))) * Src1

    def _ref(in0, in1, s0, s1, imm2):
        x = np.asarray(in0, np.float32)
        tt = x * x
        return (x * (s0 + tt * (s1 + tt * imm2))
                * np.asarray(in1, np.float32)).astype(np.float32)

    spec = Spec(body=body, reference=_ref)
    op = DveOp(name, spec, subdim=False, uops_sha={})
    dve_ops.OPS.append(op)
    dve_ops.CUSTOM_DVE_SPECS[name] = spec
    dve_ops._SUB_OPCODE_FOR_NAME[name] = (
        dve_ops._CUSTOM_DVE_ROW_BASE + len(dve_ops.OPS) - 1)
    shas = {}
    for ver in ("v3", "v4"):
        s = DveOpSpec(name=name, opcode=get_dve_sub_opcode(name),
                      uops=lower(spec, ver=ver), rd1_en=True)
        shas[ver] = s.sha(ver)
    object.__setattr__(op, "uops_sha", shas)
    _TANH5_MUL = op
    return op


def build_bass():
    import concourse.bass as bass  # noqa: F401
    import concourse.tile as tile
    from concourse import bacc, mybir

    f32 = mybir.dt.float32
    bf16 = mybir.dt.bfloat16
    AF = mybir.ActivationFunctionType
    ALU = mybir.AluOpType
    OFF = 8  # teacher-phase layer-2 lag (decouples the two recurrence chains)
    tanh5 = _get_tanh5_mul()

    nc = bacc.Bacc("TRN2", target_bir_lowering=False, num_devices=NCORES)
    xt_d = nc.dram_tensor("xt", [KX, W * BS], bf16, kind="ExternalInput")
    wc_d = nc.dram_tensor("wconst", [HID, WCOLS], bf16, kind="ExternalInput")
    h2out_d = nc.dram_tensor("h2out", [PRED, HID, BS], bf16, kind="ExternalOutput")

    with tile.TileContext(nc) as tc:
        with (
            tc.tile_pool(name="const", bufs=1) as const,
            tc.tile_pool(name="xin", bufs=3) as xin,
            tc.tile_pool(name="h1p", bufs=OFF + 3) as h1p,
            tc.tile_pool(name="st", bufs=3) as st,
            tc.tile_pool(name="work", bufs=3) as work,
            tc.tile_pool(name="psA", bufs=3, space="PSUM") as psA,
            tc.tile_pool(name="psB", bufs=3, space="PSUM") as psB,
        ):
            wc = const.tile([HID, WCOLS], bf16, tag="wc", name="wc")
            nc.sync.dma_start(out=wc, in_=wc_d[:, :])
            wt = {
                "wi0": wc[:KX, WOFF["wi0"]:WOFF["wi0"] + G4],
                "wh0": wc[:, WOFF["wh0"]:WOFF["wh0"] + G4],
                "wi1": wc[:, WOFF["wi1"]:WOFF["wi1"] + G4],
                "wh1": wc[:, WOFF["wh1"]:WOFF["wh1"] + G4],
                "wfb": wc[:, WOFF["wfb"]:WOFF["wfb"] + G4],
                "b2m": wc[:4, WOFF["b2m"]:WOFF["b2m"] + HID],
                "bones": wc[:4, WOFF["bones"]:WOFF["bones"] + G4],
            }

            def blk(w, g):
                return w[:, g * HID:(g + 1) * HID]

            # t_ext chains: [C | Tg | Tf | Ti], bf16
            def new_tx(tag):
                t = st.tile([HID, 512], bf16, tag=tag, name=tag)
                return t

            tx1 = new_tx("tx1")
            tx2 = new_tx("tx2")
            nc.vector.memset(tx1[:, 0:128], 0.0)  # C1 = 0
            nc.vector.memset(tx2[:, 0:128], 0.0)  # C2 = 0

            def new_zero(pool, tag, dt):
                t = pool.tile([HID, BS], dt, tag=tag, name=tag)
                nc.vector.memset(t, 0.0)
                return t

            h1 = new_zero(h1p, "h1", bf16)
            h2 = new_zero(st, "h2", bf16)
            h1_hist = {-1: h1}

            # Load the sigmoid table set first (it also contains tanh), so
            # the kernel pays exactly one ACT_TABLE_LOAD.
            sig0 = work.tile([HID, BS], bf16, tag="S1", name="sig0")
            nc.scalar.activation(out=sig0, in_=wc[:, 0:BS], func=AF.Sigmoid)

            # dense back-to-back matmuls: trip the PE HAM activity window so
            # the array doesn't start cold.
            warm = psA.tile([HID, 384], f32, tag="gA", name="warm")
            for k in range(8):
                nc.tensor.matmul(warm, lhsT=wc[:, 0:HID], rhs=wc[:, 0:384],
                                 start=(k == 0), stop=(k == 7))

            xt_sb = None

            def xcol_for(t):
                nonlocal xt_sb
                if t % X_CHUNK == 0:
                    nsteps = min(X_CHUNK, W - t)
                    xt_sb = xin.tile([KX, X_CHUNK * BS], bf16, tag="xt",
                                     name="xt_sb")
                    nc.sync.dma_start(out=xt_sb[:, :nsteps * BS],
                                      in_=xt_d[:, t * BS:(t + nsteps) * BS])
                return xt_sb[:, (t % X_CHUNK) * BS:(t % X_CHUNK + 1) * BS]

            # gate-matmul groups: A = blocks (g,f,i) -> psA tile; B = o -> psB
            def mm_groups(ws_rhs, gA, gB):
                """ws_rhs: list of (weightT_ap, rhs_ap). Emits A then B."""
                n = len(ws_rhs)
                for k, (wT, rhs) in enumerate(ws_rhs):
                    for g in range(3):
                        nc.tensor.matmul(blk(gA, g), lhsT=blk(wT, g), rhs=rhs,
                                         start=(k == 0 and g == 0),
                                         stop=(k == n - 1 and g == 2))
                for k, (wT, rhs) in enumerate(ws_rhs):
                    nc.tensor.matmul(gB, lhsT=blk(wT, 3), rhs=rhs,
                                     start=(k == 0), stop=(k == n - 1))

            def mm_bias2(gA, gB):
                nc.tensor.matmul(gA, lhsT=wt["b2m"],
                                 rhs=wt["bones"][:, 0:384], start=True,
                                 stop=False)
                nc.tensor.matmul(gB, lhsT=wt["b2m"],
                                 rhs=wt["bones"][:, 384:512], start=True,
                                 stop=False)

            def cell(gA, gB, tx_cur, tx_next, hpool, tag):
                # sigma(o) first: its ACT tick is then covered by the tanh
                # wait that uv already carries, so the custom h-op needs no
                # extra semaphore wait on the scalar engine.
                S = work.tile([HID, BS], bf16, tag=f"S{tag}", name=f"S{tag}")
                nc.scalar.activation(out=S, in_=gB, func=AF.Sigmoid,
                                     scale=2.0)
                nc.scalar.activation(out=tx_cur[:, 128:512], in_=gA,
                                     func=AF.Tanh)
                uv = work.tile([HID, 256], bf16, tag=f"uv{tag}",
                               name=f"uv{tag}")
                nc.vector.scalar_tensor_tensor(
                    out=uv, in0=tx_cur[:, 256:512], scalar=1.0,
                    in1=tx_cur[:, 0:256], op0=ALU.add, op1=ALU.mult)
                nc.vector.scalar_tensor_tensor(
                    out=tx_next[:, 0:128], in0=uv[:, 0:128], scalar=0.5,
                    in1=uv[:, 128:256], op0=ALU.mult, op1=ALU.add)
                h_new = hpool.tile([HID, BS], bf16, tag=f"h{tag}",
                                   name=f"h{tag}")
                nc.vector._custom_dve(tanh5, out=h_new,
                                      in0=tx_next[:, 0:128], in1=S,
                                      s0=P0, s1=P1, imm2=P2)
                return h_new

            # ---------------- teacher phase: L1 stream + L2 stream (lag OFF)
            for i in range(SEQ + OFF):
                j = i - OFF
                if j < 0:
                    # keep the PE HAM window busy until the L2 stream exists
                    wtile = psA.tile([HID, 384], f32, tag="gA", name="wtile")
                    for k in range(6):
                        nc.tensor.matmul(wtile, lhsT=wc[:, 0:HID],
                                         rhs=wc[:, 0:384], start=(k == 0),
                                         stop=(k == 5))
                g2A = g2B = None
                if 0 <= j:
                    g2A = psA.tile([HID, 384], f32, tag="gA", name="g2A")
                    g2B = psB.tile([HID, BS], f32, tag="gB", name="g2B")
                    mm_bias2(g2A, g2B)
                    ws = [(wt["wi1"], h1_hist[j]), (wt["wh1"], h2)]
                    n = len(ws)
                    for k, (wT, rhs) in enumerate(ws):
                        for g in range(3):
                            nc.tensor.matmul(blk(g2A, g), lhsT=blk(wT, g),
                                             rhs=rhs, start=False,
                                             stop=(k == n - 1 and g == 2))
                    for k, (wT, rhs) in enumerate(ws):
                        nc.tensor.matmul(g2B, lhsT=blk(wT, 3), rhs=rhs,
                                         start=False, stop=(k == n - 1))
                g1A = g1B = None
                if i < SEQ:
                    xcol = xcol_for(i)
                    g1A = psA.tile([HID, 384], f32, tag="gA", name="g1A")
                    g1B = psB.tile([HID, BS], f32, tag="gB", name="g1B")
                    mm_groups([(wt["wi0"], xcol), (wt["wh0"], h1_hist[i - 1])],
                              g1A, g1B)
                if g1A is not None:
                    tx1n = new_tx("tx1")
                    h1_hist[i] = cell(g1A, g1B, tx1, tx1n, h1p, "1")
                    tx1 = tx1n
                    h1_hist.pop(i - OFF - 1, None)
                if g2A is not None:
                    tx2n = new_tx("tx2")
                    h2 = cell(g2A, g2B, tx2, tx2n, st, "2")
                    tx2 = tx2n

            # ---------------- prediction phase: serial, hoisted issue order
            h1 = h1_hist[SEQ - 1]
            for t in range(SEQ, W):
                xcol = xcol_for(t)
                g1A = psA.tile([HID, 384], f32, tag="gA", name="g1A")
                g1B = psB.tile([HID, BS], f32, tag="gB", name="g1B")
                mm_groups([(wt["wi0"], xcol), (wt["wh0"], h1),
                           (wt["wfb"], h2)], g1A, g1B)
                g2A = psA.tile([HID, 384], f32, tag="gA", name="g2A")
                g2B = psB.tile([HID, BS], f32, tag="gB", name="g2B")
                mm_bias2(g2A, g2B)
                for g in range(3):
                    nc.tensor.matmul(blk(g2A, g), lhsT=blk(wt["wh1"], g),
                                     rhs=h2, start=False, stop=False)
                nc.tensor.matmul(g2B, lhsT=blk(wt["wh1"], 3), rhs=h2,
                                 start=False, stop=False)
                tx1n = new_tx("tx1")
                h1 = cell(g1A, g1B, tx1, tx1n, h1p, "1")
                tx1 = tx1n
                for g in range(3):
                    nc.tensor.matmul(blk(g2A, g), lhsT=blk(wt["wi1"], g),
                                     rhs=h1, start=False, stop=(g == 2))
                nc.tensor.matmul(g2B, lhsT=blk(wt["wi1"], 3), rhs=h1,
                                 start=False, stop=True)
                tx2n = new_tx("tx2")
                h2 = cell(g2A, g2B, tx2, tx2n, st, "2")
                tx2 = tx2n
                nc.sync.dma_start(out=h2out_d[t - SEQ], in_=h2)
    nc.compile()
    return nc


_BASS_CACHE = {}


def _get_bass():
    if "nc" not in _BASS_CACHE:
        _BASS_CACHE["nc"] = build_bass()
    return _BASS_CACHE["nc"]


def run(inputs, trace=False):
    """Returns (output, BassKernelResults)."""
    from concourse.bass_utils import run_bass_kernel_spmd

    prep = host_prep(inputs)
    nc = _get_bass()
    in_maps = [{"xt": prep["xt_cores"][c], "wconst": prep["wconst"]}
               for c in range(NCORES)]
    res = run_bass_kernel_spmd(nc, in_maps, core_ids=list(range(NCORES)),
                               trace=trace)
    h2_cores = [r["h2out"] for r in res.results]
    return host_post(h2_cores, prep), res


def kernel(**inputs) -> np.ndarray:
    out, _ = run(inputs, trace=False)
    return out


# revision 11
# speedup vs baseline: 1.2164x; 1.2164x over previous
"""DeepAR (2-layer LSTM, B=1024, W=288, H=128) forward on 8 Trainium2 cores.

Pure data-parallel: batch 1024 -> 128 per core; weights replicated.

Device layout is "transposed activations": every on-chip tensor is
(feature_dim = partitions, batch = free).  Cell math per step, with the
tanh identity sigmoid(x) = (tanh(x/2)+1)/2 and states C = 2c, H = 2h
(weights consuming h are pre-halved; i/f gate rows pre-halved, o rows
pre-halved for the sigmoid's scale=2, g rows full):

    psum A = [g|f|i] gate blocks (one bank), psum B = [o] (another bank)
    t_ext[:,128:512] = tanh(A)                  (ACT, 384 wide)
    S              = sigmoid(2 * B) = (To+1)/2  (ACT, 128 wide, off-chain)
    [v|u]          = (t_ext[:,256:512] + 1) * t_ext[:,0:256]   (DVE stt)
    C_new          = 0.5*v + u  -> next t_ext[:,0:128]         (DVE stt)
    H_new          = S * C_new * poly(C_new^2)                 (custom DVE op)

where poly is a degree-5 odd minimax fit of 2*tanh(0.5*X) on |X|<=2.2
(|C| stays under ~1.75 for this model; checked against the reference).
The custom DVE op fuses tanh(c) and the output-gate multiply into one
vector instruction, so each cell touches the scalar engine only once on
the critical path.

t_ext layout (bf16, 512 cols): [C_prev | Tg | Tf | Ti]; the C slot of the
NEXT step's tile is written by this step's c-op, which makes the uv
operand [C|Tg] a single contiguous access pattern.

Prediction-phase feedback (prev_y = mean_{t-1}) is folded into the
recurrence as a rank-1 matrix Wfb = Wi0[:,0] (x) (0.5*meanW) applied to
H2; means are computed on the host from the exported H2 states.
"""

import ml_dtypes
import numpy as np

BF16 = ml_dtypes.bfloat16

B = 1024
SEQ, PRED = 192, 96
W = SEQ + PRED  # 288
HID = 128
NCORES = 8
BS = B // NCORES  # 128
IN = 67
KX = IN + 2  # + ones row (bias1) + indicator row (pred feedback bias)
G4 = 4 * HID  # 512
# torch gate order (i, f, g, o) -> device order (g, f, i, o)
GATE_PERM = [2, 1, 0, 3]
X_CHUNK = 16  # scan steps per input-DMA chunk
WOFF = {"wi0": 0, "wh0": 512, "wi1": 1024, "wh1": 1536, "wfb": 2048,
        "b2m": 2560, "bones": 2688}
WCOLS = 2688 + 512  # 3200

# degree-5 odd minimax of 2*tanh(0.5*X) ~= X*(P0 + P1*X^2 + P2*X^4), |X|<=2.2
P0, P1, P2 = 0.99558505, -0.07501307, 0.0040895


def _perm_rows(w):
    """(4H, X) or (4H,) -> gate-permuted; f/i/o rows halved (tanh trick)."""
    w = w.reshape(4, HID, -1) if w.ndim == 2 else w.reshape(4, HID, 1)
    w = w[GATE_PERM].astype(np.float64).copy()  # (g, f, i, o)
    w[1] *= 0.5  # f
    w[2] *= 0.5  # i
    w[3] *= 0.5  # o
    return w  # (4, HID, X)


def _as_blocksT(w4):
    """(4, HID, K) -> (K, 4*HID) with gate blocks along columns (lhsT form)."""
    k = w4.shape[2]
    out = np.zeros((k, G4), np.float64)
    for g in range(4):
        out[:, g * HID:(g + 1) * HID] = w4[g].T
    return out


def host_prep(inputs):
    """All data-movement-only preprocessing + weight folding. Returns dict."""
    f32 = np.float32
    ge = np.asarray(inputs["given_enc"], f32)
    x_enc = np.asarray(inputs["x_enc"], f32)
    xm = np.asarray(inputs["x_mark_enc"], f32)
    mx = np.asarray(inputs["meta_x"], f32)
    tembs = [np.asarray(inputs[f"time_emb{i}"], f32) for i in range(3)]
    membs = [np.asarray(inputs[f"meta_emb{i}"], f32) for i in range(2)]

    tcat = ge[:, :, 4:7].astype(np.int32)
    time_feat = np.concatenate(
        [ge[:, :, :4]] + [tembs[i][tcat[:, :, i]] for i in range(3)], axis=-1
    )  # (B, W, 28)
    mcat = mx[:, 2:4].astype(np.int32)
    meta_feat = np.concatenate(
        [mx[:, :2]] + [membs[i][mcat[:, i]] for i in range(2)], axis=-1
    )  # (B, 34)

    nm = x_enc.mean(axis=1, keepdims=True)  # (B,1,1)
    xc = x_enc - nm
    ns = np.sqrt(xc.var(axis=1, keepdims=True) + 1e-5)
    xn = (xc / ns).astype(f32)  # (B, SEQ, 1)

    teacher = np.zeros((B, W, 1), f32)
    teacher[:, 0] = xn[:, 0]
    teacher[:, 1:SEQ] = xn[:, : SEQ - 1]
    ones = np.ones((B, W, 1), f32)
    ind = np.zeros((B, W, 1), f32)
    ind[:, SEQ:] = 1.0
    xfeat = np.concatenate(
        [teacher, time_feat, xm,
         np.broadcast_to(meta_feat[:, None, :], (B, W, 34)), ones, ind],
        axis=-1,
    )  # (B, W, 69)

    Wi0 = np.asarray(inputs["W_ih0"], np.float64)  # (512, 67)
    Wh0 = np.asarray(inputs["W_hh0"], np.float64)
    Wi1 = np.asarray(inputs["W_ih1"], np.float64)
    Wh1 = np.asarray(inputs["W_hh1"], np.float64)
    b1 = np.asarray(inputs["b_ih0"], np.float64) + np.asarray(inputs["b_hh0"], np.float64)
    b2 = np.asarray(inputs["b_ih1"], np.float64) + np.asarray(inputs["b_hh1"], np.float64)
    meanW = np.asarray(inputs["mean_W"], np.float64)  # (1, 128)
    mean_b = float(np.asarray(inputs["mean_b"]).reshape(()))

    wfb_full = Wi0[:, 0:1] @ (0.5 * meanW)  # consumes H2 = 2*h2
    bias_fb = Wi0[:, 0] * mean_b  # (512,)

    wi0T = _as_blocksT(_perm_rows(Wi0))  # (67, 512)
    wi0T_aug = np.zeros((KX, G4), np.float64)
    wi0T_aug[:IN] = wi0T
    wi0T_aug[IN] = _as_blocksT(_perm_rows(b1)).reshape(G4)  # ones row: bias1
    wi0T_aug[IN + 1] = _as_blocksT(_perm_rows(bias_fb)).reshape(G4)  # indicator
    wh0T = _as_blocksT(_perm_rows(Wh0) * 0.5)  # *0.5: h state is H = 2h
    wi1T = _as_blocksT(_perm_rows(Wi1) * 0.5)
    wh1T = _as_blocksT(_perm_rows(Wh1) * 0.5)
    wfbT = _as_blocksT(_perm_rows(wfb_full))  # (128, 512)

    b2m = _perm_rows(b2).reshape(4, HID)
    bones = np.zeros((4, G4), f32)
    for g in range(4):
        bones[g, g * HID:(g + 1) * HID] = 1.0

    # per-core transposed inputs: (KX, W*BS), feature on partitions
    xt_cores = []
    for c in range(NCORES):
        xf = xfeat[c * BS:(c + 1) * BS]  # (BS, W, KX)
        xt = np.ascontiguousarray(xf.transpose(2, 1, 0)).reshape(KX, W * BS)
        xt_cores.append(xt.astype(BF16))

    # Pack every weight into one (128, WCOLS) tensor -> single DMA.
    wconst = np.zeros((HID, WCOLS), BF16)
    wconst[:KX, WOFF["wi0"]:WOFF["wi0"] + G4] = wi0T_aug
    wconst[:, WOFF["wh0"]:WOFF["wh0"] + G4] = wh0T
    wconst[:, WOFF["wi1"]:WOFF["wi1"] + G4] = wi1T
    wconst[:, WOFF["wh1"]:WOFF["wh1"] + G4] = wh1T
    wconst[:, WOFF["wfb"]:WOFF["wfb"] + G4] = wfbT
    wconst[:4, WOFF["b2m"]:WOFF["b2m"] + HID] = b2m
    wconst[:4, WOFF["bones"]:WOFF["bones"] + G4] = bones

    return dict(
        xt_cores=xt_cores,
        wconst=wconst,
        weights=dict(
            wi0=wi0T_aug.astype(f32), wh0=wh0T.astype(f32),
            wi1=wi1T.astype(f32), wh1=wh1T.astype(f32),
            wfb=wfbT.astype(f32), b2m=b2m.astype(f32), bones=bones,
        ),
        meanW_h=(0.5 * meanW).astype(f32), mean_b=mean_b,
        norm_std=ns.astype(f32), norm_mean=nm.astype(f32),
    )


def host_post(h2_cores, prep):
    """h2_cores: list of (PRED, HID, BS) arrays of H2=2*h2. -> (B, PRED, 1)."""
    meanW_h = prep["meanW_h"][0]  # (HID,)
    out = np.empty((B, PRED, 1), np.float32)
    for c, h2 in enumerate(h2_cores):
        mn = np.einsum("h,thb->bt", meanW_h, h2.astype(np.float32)) + prep["mean_b"]
        out[c * BS:(c + 1) * BS, :, 0] = mn
    out = out * prep["norm_std"] + prep["norm_mean"]
    return out.astype(np.float32)


_TANH5_MUL = None


def _get_tanh5_mul():
    """Register the fused (sigmoid-gate * poly-tanh) custom DVE op.

    out = in0 * (s0 + t*(s1 + t*imm2)) * in1, t = in0^2.  Registered via the
    documented dve_ops extension point (OPS list + derived tables)."""
    global _TANH5_MUL
    if _TANH5_MUL is not None:
        return _TANH5_MUL
    import concourse.dve_ops as dve_ops
    from concourse.dve_ops import DveOp, get_dve_sub_opcode
    from concourse.dve_spec import Spec, Src0, Src1, C0, C1, C2, sq, lower
    from concourse.dve_uop import DveOpSpec

    name = "TANH5_MUL"
    for existing in dve_ops.OPS:
        if existing.name == name:
            _TANH5_MUL = existing
            return existing
    t = sq(Src0)
    body = (Src0 * (C0 + t * (C1 + t * C2))) * Src1

    def _ref(in0, in1, s0, s1, imm2):
        x = np.asarray(in0, np.float32)
        tt = x * x
        return (x * (s0 + tt * (s1 + tt * imm2))
                * np.asarray(in1, np.float32)).astype(np.float32)

    spec = Spec(body=body, reference=_ref)
    op = DveOp(name, spec, subdim=False, uops_sha={})
    dve_ops.OPS.append(op)
    dve_ops.CUSTOM_DVE_SPECS[name] = spec
    dve_ops._SUB_OPCODE_FOR_NAME[name] = (
        dve_ops._CUSTOM_DVE_ROW_BASE + len(dve_ops.OPS) - 1)
    shas = {}
    for ver in ("v3", "v4"):
        s = DveOpSpec(name=name, opcode=get_dve_sub_opcode(name),
                      uops=lower(spec, ver=ver), rd1_en=True)
        shas[ver] = s.sha(ver)
    object.__setattr__(op, "uops_sha", shas)
    _TANH5_MUL = op
    return op


def build_bass():
    import concourse.bass as bass  # noqa: F401
    import concourse.tile as tile
    from concourse import bacc, mybir

    f32 = mybir.dt.float32
    bf16 = mybir.dt.bfloat16
    AF = mybir.ActivationFunctionType
    ALU = mybir.AluOpType
    OFF = 8  # teacher-phase layer-2 lag (decouples the two recurrence chains)
    tanh5 = _get_tanh5_mul()

    nc = bacc.Bacc("TRN2", target_bir_lowering=False, num_devices=NCORES)
    xt_d = nc.dram_tensor("xt", [KX, W * BS], bf16, kind="ExternalInput")
    wc_d = nc.dram_tensor("wconst", [HID, WCOLS], bf16, kind="ExternalInput")
    h2out_d = nc.dram_tensor("h2out", [PRED, HID, BS], bf16, kind="ExternalOutput")

    with tile.TileContext(nc) as tc:
        with (
            tc.tile_pool(name="const", bufs=1) as const,
            tc.tile_pool(name="xin", bufs=3) as xin,
            tc.tile_pool(name="h1p", bufs=OFF + 3) as h1p,
            tc.tile_pool(name="st", bufs=3) as st,
            tc.tile_pool(name="work", bufs=3) as work,
            tc.tile_pool(name="psA", bufs=3, space="PSUM") as psA,
            tc.tile_pool(name="psB", bufs=3, space="PSUM") as psB,
        ):
            wc = const.tile([HID, WCOLS], bf16, tag="wc", name="wc")
            nc.sync.dma_start(out=wc, in_=wc_d[:, :])
            wt = {
                "wi0": wc[:KX, WOFF["wi0"]:WOFF["wi0"] + G4],
                "wh0": wc[:, WOFF["wh0"]:WOFF["wh0"] + G4],
                "wi1": wc[:, WOFF["wi1"]:WOFF["wi1"] + G4],
                "wh1": wc[:, WOFF["wh1"]:WOFF["wh1"] + G4],
                "wfb": wc[:, WOFF["wfb"]:WOFF["wfb"] + G4],
                "b2m": wc[:4, WOFF["b2m"]:WOFF["b2m"] + HID],
                "bones": wc[:4, WOFF["bones"]:WOFF["bones"] + G4],
            }

            def blk(w, g):
                return w[:, g * HID:(g + 1) * HID]

            # t_ext chains: [C | Tg | Tf | Ti], bf16
            def new_tx(tag):
                t = st.tile([HID, 512], bf16, tag=tag, name=tag)
                return t

            tx1 = new_tx("tx1")
            tx2 = new_tx("tx2")
            nc.vector.memset(tx1[:, 0:128], 0.0)  # C1 = 0
            nc.vector.memset(tx2[:, 0:128], 0.0)  # C2 = 0

            def new_zero(pool, tag, dt):
                t = pool.tile([HID, BS], dt, tag=tag, name=tag)
                nc.vector.memset(t, 0.0)
                return t

            h1 = new_zero(h1p, "h1", bf16)
            h2 = new_zero(st, "h2", bf16)
            h1_hist = {-1: h1}

            # Load the sigmoid table set first (it also contains tanh), so
            # the kernel pays exactly one ACT_TABLE_LOAD.
            sig0 = work.tile([HID, BS], bf16, tag="S1", name="sig0")
            nc.scalar.activation(out=sig0, in_=wc[:, 0:BS], func=AF.Sigmoid)

            # dense back-to-back matmuls: trip the PE HAM activity window so
            # the array doesn't start cold.
            warm = psA.tile([HID, 384], f32, tag="gA", name="warm")
            for k in range(8):
                nc.tensor.matmul(warm, lhsT=wc[:, 0:HID], rhs=wc[:, 0:384],
                                 start=(k == 0), stop=(k == 7))

            xt_sb = None

            def xcol_for(t):
                nonlocal xt_sb
                if t % X_CHUNK == 0:
                    nsteps = min(X_CHUNK, W - t)
                    xt_sb = xin.tile([KX, X_CHUNK * BS], bf16, tag="xt",
                                     name="xt_sb")
                    nc.sync.dma_start(out=xt_sb[:, :nsteps * BS],
                                      in_=xt_d[:, t * BS:(t + nsteps) * BS])
                return xt_sb[:, (t % X_CHUNK) * BS:(t % X_CHUNK + 1) * BS]

            # gate-matmul groups: A = blocks (g,f,i) -> psA tile; B = o -> psB
            def mm_groups(ws_rhs, gA, gB):
                """ws_rhs: list of (weightT_ap, rhs_ap). B (o-gate) mms are
                emitted first so sigma(o) can run on ACT while the A mms
                stream, keeping tanh unblocked right after the A tail."""
                n = len(ws_rhs)
                for k, (wT, rhs) in enumerate(ws_rhs):
                    nc.tensor.matmul(gB, lhsT=blk(wT, 3), rhs=rhs,
                                     start=(k == 0), stop=(k == n - 1))
                for k, (wT, rhs) in enumerate(ws_rhs):
                    for g in range(3):
                        nc.tensor.matmul(blk(gA, g), lhsT=blk(wT, g), rhs=rhs,
                                         start=(k == 0 and g == 0),
                                         stop=(k == n - 1 and g == 2))

            def mm_bias2(gA, gB):
                nc.tensor.matmul(gB, lhsT=wt["b2m"],
                                 rhs=wt["bones"][:, 384:512], start=True,
                                 stop=False)
                nc.tensor.matmul(gA, lhsT=wt["b2m"],
                                 rhs=wt["bones"][:, 0:384], start=True,
                                 stop=False)

            def cell(gA, gB, tx_cur, tx_next, hpool, tag):
                # sigma(o) first: its ACT tick is then covered by the tanh
                # wait that uv already carries, so the custom h-op needs no
                # extra semaphore wait on the scalar engine.
                S = work.tile([HID, BS], bf16, tag=f"S{tag}", name=f"S{tag}")
                nc.scalar.activation(out=S, in_=gB, func=AF.Sigmoid,
                                     scale=2.0)
                nc.scalar.activation(out=tx_cur[:, 128:512], in_=gA,
                                     func=AF.Tanh)
                uv = work.tile([HID, 256], bf16, tag=f"uv{tag}",
                               name=f"uv{tag}")
                nc.vector.scalar_tensor_tensor(
                    out=uv, in0=tx_cur[:, 256:512], scalar=1.0,
                    in1=tx_cur[:, 0:256], op0=ALU.add, op1=ALU.mult)
                nc.vector.scalar_tensor_tensor(
                    out=tx_next[:, 0:128], in0=uv[:, 0:128], scalar=0.5,
                    in1=uv[:, 128:256], op0=ALU.mult, op1=ALU.add)
                h_new = hpool.tile([HID, BS], bf16, tag=f"h{tag}",
                                   name=f"h{tag}")
                nc.vector._custom_dve(tanh5, out=h_new,
                                      in0=tx_next[:, 0:128], in1=S,
                                      s0=P0, s1=P1, imm2=P2)
                return h_new

            # ---------------- teacher phase: L1 stream + L2 stream (lag OFF)
            for i in range(SEQ + OFF):
                j = i - OFF
                if j < 0:
                    # keep the PE HAM window busy until the L2 stream exists
                    wtile = psA.tile([HID, 384], f32, tag="gA", name="wtile")
                    for k in range(6):
                        nc.tensor.matmul(wtile, lhsT=wc[:, 0:HID],
                                         rhs=wc[:, 0:384], start=(k == 0),
                                         stop=(k == 5))
                g2A = g2B = None
                if 0 <= j:
                    g2A = psA.tile([HID, 384], f32, tag="gA", name="g2A")
                    g2B = psB.tile([HID, BS], f32, tag="gB", name="g2B")
                    mm_bias2(g2A, g2B)
                    ws = [(wt["wi1"], h1_hist[j]), (wt["wh1"], h2)]
                    n = len(ws)
                    for k, (wT, rhs) in enumerate(ws):
                        nc.tensor.matmul(g2B, lhsT=blk(wT, 3), rhs=rhs,
                                         start=False, stop=(k == n - 1))
                    for k, (wT, rhs) in enumerate(ws):
                        for g in range(3):
                            nc.tensor.matmul(blk(g2A, g), lhsT=blk(wT, g),
                                             rhs=rhs, start=False,
                                             stop=(k == n - 1 and g == 2))
                g1A = g1B = None
                if i < SEQ:
                    xcol = xcol_for(i)
                    g1A = psA.tile([HID, 384], f32, tag="gA", name="g1A")
                    g1B = psB.tile([HID, BS], f32, tag="gB", name="g1B")
                    mm_groups([(wt["wi0"], xcol), (wt["wh0"], h1_hist[i - 1])],
                              g1A, g1B)
                if g2A is not None:
                    tx2n = new_tx("tx2")
                    h2 = cell(g2A, g2B, tx2, tx2n, st, "2")
                    tx2 = tx2n
                if g1A is not None:
                    tx1n = new_tx("tx1")
                    h1_hist[i] = cell(g1A, g1B, tx1, tx1n, h1p, "1")
                    tx1 = tx1n
                    h1_hist.pop(i - OFF - 1, None)

            # ---------------- prediction phase: serial, hoisted issue order
            h1 = h1_hist[SEQ - 1]
            for t in range(SEQ, W):
                xcol = xcol_for(t)
                g1A = psA.tile([HID, 384], f32, tag="gA", name="g1A")
                g1B = psB.tile([HID, BS], f32, tag="gB", name="g1B")
                mm_groups([(wt["wi0"], xcol), (wt["wh0"], h1),
                           (wt["wfb"], h2)], g1A, g1B)
                g2A = psA.tile([HID, 384], f32, tag="gA", name="g2A")
                g2B = psB.tile([HID, BS], f32, tag="gB", name="g2B")
                mm_bias2(g2A, g2B)
                nc.tensor.matmul(g2B, lhsT=blk(wt["wh1"], 3), rhs=h2,
                                 start=False, stop=False)
                for g in range(3):
                    nc.tensor.matmul(blk(g2A, g), lhsT=blk(wt["wh1"], g),
                                     rhs=h2, start=False, stop=False)
                tx1n = new_tx("tx1")
                h1 = cell(g1A, g1B, tx1, tx1n, h1p, "1")
                tx1 = tx1n
                nc.tensor.matmul(g2B, lhsT=blk(wt["wi1"], 3), rhs=h1,
                                 start=False, stop=True)
                for g in range(3):
                    nc.tensor.matmul(blk(g2A, g), lhsT=blk(wt["wi1"], g),
                                     rhs=h1, start=False, stop=(g == 2))
                tx2n = new_tx("tx2")
                h2 = cell(g2A, g2B, tx2, tx2n, st, "2")
                tx2 = tx2n
                nc.sync.dma_start(out=h2out_d[t - SEQ], in_=h2)
    nc.compile()
    return nc


_BASS_CACHE = {}


def _get_bass():
    if "nc" not in _BASS_CACHE:
        _BASS_CACHE["nc"] = build_bass()
    return _BASS_CACHE["nc"]


def run(inputs, trace=False):
    """Returns (output, BassKernelResults)."""
    from concourse.bass_utils import run_bass_kernel_spmd

    prep = host_prep(inputs)
    nc = _get_bass()
    in_maps = [{"xt": prep["xt_cores"][c], "wconst": prep["wconst"]}
               for c in range(NCORES)]
    res = run_bass_kernel_spmd(nc, in_maps, core_ids=list(range(NCORES)),
                               trace=trace)
    h2_cores = [r["h2out"] for r in res.results]
    return host_post(h2_cores, prep), res


def kernel(**inputs) -> np.ndarray:
    out, _ = run(inputs, trace=False)
    return out


# revision 18
# speedup vs baseline: 1.2165x; 1.0001x over previous
"""DeepAR (2-layer LSTM, B=1024, W=288, H=128) forward on 8 Trainium2 cores.

Pure data-parallel: batch 1024 -> 128 per core; weights replicated.

Device layout is "transposed activations": every on-chip tensor is
(feature_dim = partitions, batch = free).  Cell math per step, with the
tanh identity sigmoid(x) = (tanh(x/2)+1)/2 and states C = 2c, H = 2h
(weights consuming h are pre-halved; i/f gate rows pre-halved, o rows
pre-halved for the sigmoid's scale=2, g rows full):

    psum A = [g|f|i] gate blocks (one bank), psum B = [o] (another bank)
    t_ext[:,128:512] = tanh(A)                  (ACT, 384 wide)
    S              = sigmoid(2 * B) = (To+1)/2  (ACT, 128 wide, off-chain)
    [v|u]          = (t_ext[:,256:512] + 1) * t_ext[:,0:256]   (DVE stt)
    C_new          = 0.5*v + u  -> next t_ext[:,0:128]         (DVE stt)
    H_new          = S * C_new * poly(C_new^2)                 (custom DVE op)

where poly is a degree-5 odd minimax fit of 2*tanh(0.5*X) on |X|<=2.2
(|C| stays under ~1.75 for this model; checked against the reference).
The custom DVE op fuses tanh(c) and the output-gate multiply into one
vector instruction, so each cell touches the scalar engine only once on
the critical path.

t_ext layout (bf16, 512 cols): [C_prev | Tg | Tf | Ti]; the C slot of the
NEXT step's tile is written by this step's c-op, which makes the uv
operand [C|Tg] a single contiguous access pattern.

Prediction-phase feedback (prev_y = mean_{t-1}) is folded into the
recurrence as a rank-1 matrix Wfb = Wi0[:,0] (x) (0.5*meanW) applied to
H2; means are computed on the host from the exported H2 states.
"""

import ml_dtypes
import numpy as np

BF16 = ml_dtypes.bfloat16

B = 1024
SEQ, PRED = 192, 96
W = SEQ + PRED  # 288
HID = 128
NCORES = 8
BS = B // NCORES  # 128
IN = 67
KX = IN + 2  # + ones row (bias1) + indicator row (pred feedback bias)
G4 = 4 * HID  # 512
# torch gate order (i, f, g, o) -> device order (g, f, i, o)
GATE_PERM = [2, 1, 0, 3]
X_CHUNK = 16  # scan steps per input-DMA chunk
WOFF = {"wi0": 0, "wh0": 512, "wi1": 1024, "wh1": 1536, "wfb": 2048,
        "b2m": 2560, "bones": 2688}
WCOLS = 2688 + 512  # 3200

# degree-5 odd minimax of 2*tanh(0.5*X) ~= X*(P0 + P1*X^2 + P2*X^4), |X|<=2.2
P0, P1, P2 = 0.99558505, -0.07501307, 0.0040895


def _perm_rows(w):
    """(4H, X) or (4H,) -> gate-permuted; f/i/o rows halved (tanh trick)."""
    w = w.reshape(4, HID, -1) if w.ndim == 2 else w.reshape(4, HID, 1)
    w = w[GATE_PERM].astype(np.float64).copy()  # (g, f, i, o)
    w[1] *= 0.5  # f
    w[2] *= 0.5  # i
    w[3] *= 0.5  # o
    return w  # (4, HID, X)


def _as_blocksT(w4):
    """(4, HID, K) -> (K, 4*HID) with gate blocks along columns (lhsT form)."""
    k = w4.shape[2]
    out = np.zeros((k, G4), np.float64)
    for g in range(4):
        out[:, g * HID:(g + 1) * HID] = w4[g].T
    return out


def host_prep(inputs):
    """All data-movement-only preprocessing + weight folding. Returns dict."""
    f32 = np.float32
    ge = np.asarray(inputs["given_enc"], f32)
    x_enc = np.asarray(inputs["x_enc"], f32)
    xm = np.asarray(inputs["x_mark_enc"], f32)
    mx = np.asarray(inputs["meta_x"], f32)
    tembs = [np.asarray(inputs[f"time_emb{i}"], f32) for i in range(3)]
    membs = [np.asarray(inputs[f"meta_emb{i}"], f32) for i in range(2)]

    tcat = ge[:, :, 4:7].astype(np.int32)
    time_feat = np.concatenate(
        [ge[:, :, :4]] + [tembs[i][tcat[:, :, i]] for i in range(3)], axis=-1
    )  # (B, W, 28)
    mcat = mx[:, 2:4].astype(np.int32)
    meta_feat = np.concatenate(
        [mx[:, :2]] + [membs[i][mcat[:, i]] for i in range(2)], axis=-1
    )  # (B, 34)

    nm = x_enc.mean(axis=1, keepdims=True)  # (B,1,1)
    xc = x_enc - nm
    ns = np.sqrt(xc.var(axis=1, keepdims=True) + 1e-5)
    xn = (xc / ns).astype(f32)  # (B, SEQ, 1)

    teacher = np.zeros((B, W, 1), f32)
    teacher[:, 0] = xn[:, 0]
    teacher[:, 1:SEQ] = xn[:, : SEQ - 1]
    ones = np.ones((B, W, 1), f32)
    ind = np.zeros((B, W, 1), f32)
    ind[:, SEQ:] = 1.0
    xfeat = np.concatenate(
        [teacher, time_feat, xm,
         np.broadcast_to(meta_feat[:, None, :], (B, W, 34)), ones, ind],
        axis=-1,
    )  # (B, W, 69)

    Wi0 = np.asarray(inputs["W_ih0"], np.float64)  # (512, 67)
    Wh0 = np.asarray(inputs["W_hh0"], np.float64)
    Wi1 = np.asarray(inputs["W_ih1"], np.float64)
    Wh1 = np.asarray(inputs["W_hh1"], np.float64)
    b1 = np.asarray(inputs["b_ih0"], np.float64) + np.asarray(inputs["b_hh0"], np.float64)
    b2 = np.asarray(inputs["b_ih1"], np.float64) + np.asarray(inputs["b_hh1"], np.float64)
    meanW = np.asarray(inputs["mean_W"], np.float64)  # (1, 128)
    mean_b = float(np.asarray(inputs["mean_b"]).reshape(()))

    wfb_full = Wi0[:, 0:1] @ (0.5 * meanW)  # consumes H2 = 2*h2
    bias_fb = Wi0[:, 0] * mean_b  # (512,)

    wi0T = _as_blocksT(_perm_rows(Wi0))  # (67, 512)
    wi0T_aug = np.zeros((KX, G4), np.float64)
    wi0T_aug[:IN] = wi0T
    wi0T_aug[IN] = _as_blocksT(_perm_rows(b1)).reshape(G4)  # ones row: bias1
    wi0T_aug[IN + 1] = _as_blocksT(_perm_rows(bias_fb)).reshape(G4)  # indicator
    wh0T = _as_blocksT(_perm_rows(Wh0) * 0.5)  # *0.5: h state is H = 2h
    wi1T = _as_blocksT(_perm_rows(Wi1) * 0.5)
    wh1T = _as_blocksT(_perm_rows(Wh1) * 0.5)
    wfbT = _as_blocksT(_perm_rows(wfb_full))  # (128, 512)

    b2m = _perm_rows(b2).reshape(4, HID)
    bones = np.zeros((4, G4), f32)
    for g in range(4):
        bones[g, g * HID:(g + 1) * HID] = 1.0

    # per-core transposed inputs: (KX, W*BS), feature on partitions
    xt_cores = []
    for c in range(NCORES):
        xf = xfeat[c * BS:(c + 1) * BS]  # (BS, W, KX)
        xt = np.ascontiguousarray(xf.transpose(2, 1, 0)).reshape(KX, W * BS)
        xt_cores.append(xt.astype(BF16))

    # Pack every weight into one (128, WCOLS) tensor -> single DMA.
    wconst = np.zeros((HID, WCOLS), BF16)
    wconst[:KX, WOFF["wi0"]:WOFF["wi0"] + G4] = wi0T_aug
    wconst[:, WOFF["wh0"]:WOFF["wh0"] + G4] = wh0T
    wconst[:, WOFF["wi1"]:WOFF["wi1"] + G4] = wi1T
    wconst[:, WOFF["wh1"]:WOFF["wh1"] + G4] = wh1T
    wconst[:, WOFF["wfb"]:WOFF["wfb"] + G4] = wfbT
    wconst[:4, WOFF["b2m"]:WOFF["b2m"] + HID] = b2m
    wconst[:4, WOFF["bones"]:WOFF["bones"] + G4] = bones

    return dict(
        xt_cores=xt_cores,
        wconst=wconst,
        weights=dict(
            wi0=wi0T_aug.astype(f32), wh0=wh0T.astype(f32),
            wi1=wi1T.astype(f32), wh1=wh1T.astype(f32),
            wfb=wfbT.astype(f32), b2m=b2m.astype(f32), bones=bones,
        ),
        meanW_h=(0.5 * meanW).astype(f32), mean_b=mean_b,
        norm_std=ns.astype(f32), norm_mean=nm.astype(f32),
    )


def host_post(h2_cores, prep):
    """h2_cores: list of (PRED, HID, BS) arrays of H2=2*h2. -> (B, PRED, 1)."""
    meanW_h = prep["meanW_h"][0]  # (HID,)
    out = np.empty((B, PRED, 1), np.float32)
    for c, h2 in enumerate(h2_cores):
        mn = np.einsum("h,thb->bt", meanW_h, h2.astype(np.float32)) + prep["mean_b"]
        out[c * BS:(c + 1) * BS, :, 0] = mn
    out = out * prep["norm_std"] + prep["norm_mean"]
    return out.astype(np.float32)


_TANH5_MUL = None


def _get_tanh5_mul():
    """Register the fused (sigmoid-gate * poly-tanh) custom DVE op.

    out = in0 * (s0 + t*(s1 + t*imm2)) * in1, t = in0^2.  Registered via the
    documented dve_ops extension point (OPS list + derived tables)."""
    global _TANH5_MUL
    if _TANH5_MUL is not None:
        return _TANH5_MUL
    import concourse.dve_ops as dve_ops
    from concourse.dve_ops import DveOp, get_dve_sub_opcode
    from concourse.dve_spec import Spec, Src0, Src1, C0, C1, C2, sq, lower
    from concourse.dve_uop import DveOpSpec

    name = "TANH5_MUL"
    for existing in dve_ops.OPS:
        if existing.name == name:
            _TANH5_MUL = existing
            return existing
    t = sq(Src0)
    body = (Src0 * (C0 + t * (C1 + t * C2))) * Src1

    def _ref(in0, in1, s0, s1, imm2):
        x = np.asarray(in0, np.float32)
        tt = x * x
        return (x * (s0 + tt * (s1 + tt * imm2))
                * np.asarray(in1, np.float32)).astype(np.float32)

    spec = Spec(body=body, reference=_ref)
    op = DveOp(name, spec, subdim=False, uops_sha={})
    dve_ops.OPS.append(op)
    dve_ops.CUSTOM_DVE_SPECS[name] = spec
    dve_ops._SUB_OPCODE_FOR_NAME[name] = (
        dve_ops._CUSTOM_DVE_ROW_BASE + len(dve_ops.OPS) - 1)
    shas = {}
    for ver in ("v3", "v4"):
        s = DveOpSpec(name=name, opcode=get_dve_sub_opcode(name),
                      uops=lower(spec, ver=ver), rd1_en=True)
        shas[ver] = s.sha(ver)
    object.__setattr__(op, "uops_sha", shas)
    _TANH5_MUL = op
    return op


def build_bass():
    import concourse.bass as bass  # noqa: F401
    import concourse.tile as tile
    from concourse import bacc, mybir

    f32 = mybir.dt.float32
    bf16 = mybir.dt.bfloat16
    AF = mybir.ActivationFunctionType
    ALU = mybir.AluOpType
    OFF = 8  # teacher-phase layer-2 lag (decouples the two recurrence chains)
    tanh5 = _get_tanh5_mul()

    nc = bacc.Bacc("TRN2", target_bir_lowering=False, num_devices=NCORES)
    xt_d = nc.dram_tensor("xt", [KX, W * BS], bf16, kind="ExternalInput")
    wc_d = nc.dram_tensor("wconst", [HID, WCOLS], bf16, kind="ExternalInput")
    h2out_d = nc.dram_tensor("h2out", [PRED, HID, BS], bf16, kind="ExternalOutput")

    with tile.TileContext(nc) as tc:
        with (
            tc.tile_pool(name="const", bufs=1) as const,
            tc.tile_pool(name="xin", bufs=3) as xin,
            tc.tile_pool(name="h1p", bufs=OFF + 3) as h1p,
            tc.tile_pool(name="st", bufs=3) as st,
            tc.tile_pool(name="work", bufs=3) as work,
            tc.tile_pool(name="psA", bufs=3, space="PSUM") as psA,
            tc.tile_pool(name="psB", bufs=3, space="PSUM") as psB,
        ):
            wc = const.tile([HID, WCOLS], bf16, tag="wc", name="wc")
            nc.sync.dma_start(out=wc, in_=wc_d[:, :])
            wt = {
                "wi0": wc[:KX, WOFF["wi0"]:WOFF["wi0"] + G4],
                "wh0": wc[:, WOFF["wh0"]:WOFF["wh0"] + G4],
                "wi1": wc[:, WOFF["wi1"]:WOFF["wi1"] + G4],
                "wh1": wc[:, WOFF["wh1"]:WOFF["wh1"] + G4],
                "wfb": wc[:, WOFF["wfb"]:WOFF["wfb"] + G4],
                "b2m": wc[:4, WOFF["b2m"]:WOFF["b2m"] + HID],
                "bones": wc[:4, WOFF["bones"]:WOFF["bones"] + G4],
            }

            def blk(w, g):
                return w[:, g * HID:(g + 1) * HID]

            # t_ext chains: [C | Tg | Tf | Ti], bf16
            def new_tx(tag):
                t = st.tile([HID, 512], bf16, tag=tag, name=tag)
                return t

            tx1 = new_tx("tx1")
            tx2 = new_tx("tx2")
            nc.vector.memset(tx1[:, 0:128], 0.0)  # C1 = 0
            nc.vector.memset(tx2[:, 0:128], 0.0)  # C2 = 0

            def new_zero(pool, tag, dt):
                t = pool.tile([HID, BS], dt, tag=tag, name=tag)
                nc.vector.memset(t, 0.0)
                return t

            h1 = new_zero(h1p, "h1", bf16)
            h2 = new_zero(st, "h2", bf16)
            h1_hist = {-1: h1}

            # Load the sigmoid table set first (it also contains tanh), so
            # the kernel pays exactly one ACT_TABLE_LOAD.
            sig0 = work.tile([HID, BS], bf16, tag="S1", name="sig0")
            nc.scalar.activation(out=sig0, in_=wc[:, 0:BS], func=AF.Sigmoid)

            # dense back-to-back matmuls: trip the PE HAM activity window so
            # the array doesn't start cold.
            warm = psA.tile([HID, 384], f32, tag="gA", name="warm")
            for k in range(8):
                nc.tensor.matmul(warm, lhsT=wc[:, 0:HID], rhs=wc[:, 0:384],
                                 start=(k == 0), stop=(k == 7))

            xt_sb = None

            def xcol_for(t):
                nonlocal xt_sb
                if t % X_CHUNK == 0:
                    nsteps = min(X_CHUNK, W - t)
                    xt_sb = xin.tile([KX, X_CHUNK * BS], bf16, tag="xt",
                                     name="xt_sb")
                    nc.sync.dma_start(out=xt_sb[:, :nsteps * BS],
                                      in_=xt_d[:, t * BS:(t + nsteps) * BS])
                return xt_sb[:, (t % X_CHUNK) * BS:(t % X_CHUNK + 1) * BS]

            # gate-matmul groups: A = blocks (g,f,i) -> psA tile; B = o -> psB
            def mm_groups(ws_rhs, gA, gB):
                """ws_rhs: list of (weightT_ap, rhs_ap). B (o-gate) mms are
                emitted first so sigma(o) can run on ACT while the A mms
                stream, keeping tanh unblocked right after the A tail."""
                n = len(ws_rhs)
                for k, (wT, rhs) in enumerate(ws_rhs):
                    nc.tensor.matmul(gB, lhsT=blk(wT, 3), rhs=rhs,
                                     start=(k == 0), stop=(k == n - 1))
                for k, (wT, rhs) in enumerate(ws_rhs):
                    for g in range(3):
                        nc.tensor.matmul(blk(gA, g), lhsT=blk(wT, g), rhs=rhs,
                                         start=(k == 0 and g == 0),
                                         stop=(k == n - 1 and g == 2))

            def mm_bias2(gA, gB):
                nc.tensor.matmul(gB, lhsT=wt["b2m"],
                                 rhs=wt["bones"][:, 384:512], start=True,
                                 stop=False)
                nc.tensor.matmul(gA, lhsT=wt["b2m"],
                                 rhs=wt["bones"][:, 0:384], start=True,
                                 stop=False)

            def cell(gA, gB, tx_cur, tx_next, hpool, tag):
                # sigma(o) first: its ACT tick is then covered by the tanh
                # wait that uv already carries, so the custom h-op needs no
                # extra semaphore wait on the scalar engine.
                S = work.tile([HID, BS], bf16, tag=f"S{tag}", name=f"S{tag}")
                nc.scalar.activation(out=S, in_=gB, func=AF.Sigmoid,
                                     scale=2.0)
                nc.scalar.activation(out=tx_cur[:, 128:512], in_=gA,
                                     func=AF.Tanh)
                uv = work.tile([HID, 256], bf16, tag=f"uv{tag}",
                               name=f"uv{tag}")
                nc.vector.scalar_tensor_tensor(
                    out=uv, in0=tx_cur[:, 256:512], scalar=1.0,
                    in1=tx_cur[:, 0:256], op0=ALU.add, op1=ALU.mult)
                nc.vector.scalar_tensor_tensor(
                    out=tx_next[:, 0:128], in0=uv[:, 0:128], scalar=0.5,
                    in1=uv[:, 128:256], op0=ALU.mult, op1=ALU.add)
                h_new = hpool.tile([HID, BS], bf16, tag=f"h{tag}",
                                   name=f"h{tag}")
                nc.vector._custom_dve(tanh5, out=h_new,
                                      in0=tx_next[:, 0:128], in1=S,
                                      s0=P0, s1=P1, imm2=P2)
                return h_new

            # ---------------- teacher phase: L1 stream + L2 stream (lag OFF)
            for i in range(SEQ + OFF):
                j = i - OFF
                if j < 0:
                    # keep the PE HAM window busy until the L2 stream exists
                    wtile = psA.tile([HID, 384], f32, tag="gA", name="wtile")
                    for k in range(6):
                        nc.tensor.matmul(wtile, lhsT=wc[:, 0:HID],
                                         rhs=wc[:, 0:384], start=(k == 0),
                                         stop=(k == 5))
                g2A = g2B = None
                if 0 <= j:
                    g2A = psA.tile([HID, 384], f32, tag="gA", name="g2A")
                    g2B = psB.tile([HID, BS], f32, tag="gB", name="g2B")
                    mm_bias2(g2A, g2B)
                    ws = [(wt["wi1"], h1_hist[j]), (wt["wh1"], h2)]
                    n = len(ws)
                    for k, (wT, rhs) in enumerate(ws):
                        nc.tensor.matmul(g2B, lhsT=blk(wT, 3), rhs=rhs,
                                         start=False, stop=(k == n - 1))
                    for k, (wT, rhs) in enumerate(ws):
                        for g in range(3):
                            nc.tensor.matmul(blk(g2A, g), lhsT=blk(wT, g),
                                             rhs=rhs, start=False,
                                             stop=(k == n - 1 and g == 2))
                g1A = g1B = None
                if i < SEQ:
                    xcol = xcol_for(i)
                    g1A = psA.tile([HID, 384], f32, tag="gA", name="g1A")
                    g1B = psB.tile([HID, BS], f32, tag="gB", name="g1B")
                    mm_groups([(wt["wi0"], xcol), (wt["wh0"], h1_hist[i - 1])],
                              g1A, g1B)
                # L1's cell is emitted first: the static per-step DVE order
                # becomes [uv1, c1, B1, uv2, c2, B2], so the L1 chain's uv
                # never queues behind L2's ops; L2's lag absorbs the wait.
                if g1A is not None:
                    tx1n = new_tx("tx1")
                    h1_hist[i] = cell(g1A, g1B, tx1, tx1n, h1p, "1")
                    tx1 = tx1n
                    h1_hist.pop(i - OFF - 1, None)
                if g2A is not None:
                    tx2n = new_tx("tx2")
                    h2 = cell(g2A, g2B, tx2, tx2n, st, "2")
                    tx2 = tx2n

            # ---------------- prediction phase: serial, hoisted issue order
            h1 = h1_hist[SEQ - 1]
            for t in range(SEQ, W):
                xcol = xcol_for(t)
                g1A = psA.tile([HID, 384], f32, tag="gA", name="g1A")
                g1B = psB.tile([HID, BS], f32, tag="gB", name="g1B")
                mm_groups([(wt["wi0"], xcol), (wt["wh0"], h1),
                           (wt["wfb"], h2)], g1A, g1B)
                g2A = psA.tile([HID, 384], f32, tag="gA", name="g2A")
                g2B = psB.tile([HID, BS], f32, tag="gB", name="g2B")
                mm_bias2(g2A, g2B)
                nc.tensor.matmul(g2B, lhsT=blk(wt["wh1"], 3), rhs=h2,
                                 start=False, stop=False)
                for g in range(3):
                    nc.tensor.matmul(blk(g2A, g), lhsT=blk(wt["wh1"], g),
                                     rhs=h2, start=False, stop=False)
                tx1n = new_tx("tx1")
                h1 = cell(g1A, g1B, tx1, tx1n, h1p, "1")
                tx1 = tx1n
                nc.tensor.matmul(g2B, lhsT=blk(wt["wi1"], 3), rhs=h1,
                                 start=False, stop=True)
                for g in range(3):
                    nc.tensor.matmul(blk(g2A, g), lhsT=blk(wt["wi1"], g),
                                     rhs=h1, start=False, stop=(g == 2))
                tx2n = new_tx("tx2")
                h2 = cell(g2A, g2B, tx2, tx2n, st, "2")
                tx2 = tx2n
                nc.sync.dma_start(out=h2out_d[t - SEQ], in_=h2)
    nc.compile()
    return nc


_BASS_CACHE = {}


def _get_bass():
    if "nc" not in _BASS_CACHE:
        _BASS_CACHE["nc"] = build_bass()
    return _BASS_CACHE["nc"]


def run(inputs, trace=False):
    """Returns (output, BassKernelResults)."""
    from concourse.bass_utils import run_bass_kernel_spmd

    prep = host_prep(inputs)
    nc = _get_bass()
    in_maps = [{"xt": prep["xt_cores"][c], "wconst": prep["wconst"]}
               for c in range(NCORES)]
    res = run_bass_kernel_spmd(nc, in_maps, core_ids=list(range(NCORES)),
                               trace=trace)
    h2_cores = [r["h2out"] for r in res.results]
    return host_post(h2_cores, prep), res


def kernel(**inputs) -> np.ndarray:
    out, _ = run(inputs, trace=False)
    return out


# revision 19
# speedup vs baseline: 1.2176x; 1.0010x over previous
"""DeepAR (2-layer LSTM, B=1024, W=288, H=128) forward on 8 Trainium2 cores.

Pure data-parallel: batch 1024 -> 128 per core; weights replicated.

Device layout is "transposed activations": every on-chip tensor is
(feature_dim = partitions, batch = free).  Cell math per step, with the
tanh identity sigmoid(x) = (tanh(x/2)+1)/2 and states C = 2c, H = 2h
(weights consuming h are pre-halved; i/f gate rows pre-halved, o rows
pre-halved for the sigmoid's scale=2, g rows full):

    psum A = [g|f|i] gate blocks (one bank), psum B = [o] (another bank)
    t_ext[:,128:512] = tanh(A)                  (ACT, 384 wide)
    S              = sigmoid(2 * B) = (To+1)/2  (ACT, 128 wide, off-chain)
    [v|u]          = (t_ext[:,256:512] + 1) * t_ext[:,0:256]   (DVE stt)
    C_new          = 0.5*v + u  -> next t_ext[:,0:128]         (DVE stt)
    H_new          = S * C_new * poly(C_new^2)                 (custom DVE op)

where poly is a degree-5 odd minimax fit of 2*tanh(0.5*X) on |X|<=2.2
(|C| stays under ~1.75 for this model; checked against the reference).
The custom DVE op fuses tanh(c) and the output-gate multiply into one
vector instruction, so each cell touches the scalar engine only once on
the critical path.

t_ext layout (bf16, 512 cols): [C_prev | Tg | Tf | Ti]; the C slot of the
NEXT step's tile is written by this step's c-op, which makes the uv
operand [C|Tg] a single contiguous access pattern.

Prediction-phase feedback (prev_y = mean_{t-1}) is folded into the
recurrence as a rank-1 matrix Wfb = Wi0[:,0] (x) (0.5*meanW) applied to
H2; means are computed on the host from the exported H2 states.
"""

import ml_dtypes
import numpy as np

BF16 = ml_dtypes.bfloat16

B = 1024
SEQ, PRED = 192, 96
W = SEQ + PRED  # 288
HID = 128
NCORES = 8
BS = B // NCORES  # 128
IN = 67
KX = IN + 2  # + ones row (bias1) + indicator row (pred feedback bias)
G4 = 4 * HID  # 512
# torch gate order (i, f, g, o) -> device order (g, f, i, o)
GATE_PERM = [2, 1, 0, 3]
X_CHUNK = 16  # scan steps per input-DMA chunk
WOFF = {"wi0": 0, "wh0": 512, "wi1": 1024, "wh1": 1536, "wfb": 2048,
        "b2m": 2560, "bones": 2688}
WCOLS = 2688 + 512  # 3200

# degree-5 odd minimax of 2*tanh(0.5*X) ~= X*(P0 + P1*X^2 + P2*X^4), |X|<=2.2
P0, P1, P2 = 0.99558505, -0.07501307, 0.0040895


def _perm_rows(w):
    """(4H, X) or (4H,) -> gate-permuted; f/i/o rows halved (tanh trick)."""
    w = w.reshape(4, HID, -1) if w.ndim == 2 else w.reshape(4, HID, 1)
    w = w[GATE_PERM].astype(np.float64).copy()  # (g, f, i, o)
    w[1] *= 0.5  # f
    w[2] *= 0.5  # i
    w[3] *= 0.5  # o
    return w  # (4, HID, X)


def _as_blocksT(w4):
    """(4, HID, K) -> (K, 4*HID) with gate blocks along columns (lhsT form)."""
    k = w4.shape[2]
    out = np.zeros((k, G4), np.float64)
    for g in range(4):
        out[:, g * HID:(g + 1) * HID] = w4[g].T
    return out


def host_prep(inputs):
    """All data-movement-only preprocessing + weight folding. Returns dict."""
    f32 = np.float32
    ge = np.asarray(inputs["given_enc"], f32)
    x_enc = np.asarray(inputs["x_enc"], f32)
    xm = np.asarray(inputs["x_mark_enc"], f32)
    mx = np.asarray(inputs["meta_x"], f32)
    tembs = [np.asarray(inputs[f"time_emb{i}"], f32) for i in range(3)]
    membs = [np.asarray(inputs[f"meta_emb{i}"], f32) for i in range(2)]

    tcat = ge[:, :, 4:7].astype(np.int32)
    time_feat = np.concatenate(
        [ge[:, :, :4]] + [tembs[i][tcat[:, :, i]] for i in range(3)], axis=-1
    )  # (B, W, 28)
    mcat = mx[:, 2:4].astype(np.int32)
    meta_feat = np.concatenate(
        [mx[:, :2]] + [membs[i][mcat[:, i]] for i in range(2)], axis=-1
    )  # (B, 34)

    nm = x_enc.mean(axis=1, keepdims=True)  # (B,1,1)
    xc = x_enc - nm
    ns = np.sqrt(xc.var(axis=1, keepdims=True) + 1e-5)
    xn = (xc / ns).astype(f32)  # (B, SEQ, 1)

    teacher = np.zeros((B, W, 1), f32)
    teacher[:, 0] = xn[:, 0]
    teacher[:, 1:SEQ] = xn[:, : SEQ - 1]
    ones = np.ones((B, W, 1), f32)
    ind = np.zeros((B, W, 1), f32)
    ind[:, SEQ:] = 1.0
    xfeat = np.concatenate(
        [teacher, time_feat, xm,
         np.broadcast_to(meta_feat[:, None, :], (B, W, 34)), ones, ind],
        axis=-1,
    )  # (B, W, 69)

    Wi0 = np.asarray(inputs["W_ih0"], np.float64)  # (512, 67)
    Wh0 = np.asarray(inputs["W_hh0"], np.float64)
    Wi1 = np.asarray(inputs["W_ih1"], np.float64)
    Wh1 = np.asarray(inputs["W_hh1"], np.float64)
    b1 = np.asarray(inputs["b_ih0"], np.float64) + np.asarray(inputs["b_hh0"], np.float64)
    b2 = np.asarray(inputs["b_ih1"], np.float64) + np.asarray(inputs["b_hh1"], np.float64)
    meanW = np.asarray(inputs["mean_W"], np.float64)  # (1, 128)
    mean_b = float(np.asarray(inputs["mean_b"]).reshape(()))

    wfb_full = Wi0[:, 0:1] @ (0.5 * meanW)  # consumes H2 = 2*h2
    bias_fb = Wi0[:, 0] * mean_b  # (512,)

    wi0T = _as_blocksT(_perm_rows(Wi0))  # (67, 512)
    wi0T_aug = np.zeros((KX, G4), np.float64)
    wi0T_aug[:IN] = wi0T
    wi0T_aug[IN] = _as_blocksT(_perm_rows(b1)).reshape(G4)  # ones row: bias1
    wi0T_aug[IN + 1] = _as_blocksT(_perm_rows(bias_fb)).reshape(G4)  # indicator
    wh0T = _as_blocksT(_perm_rows(Wh0) * 0.5)  # *0.5: h state is H = 2h
    wi1T = _as_blocksT(_perm_rows(Wi1) * 0.5)
    wh1T = _as_blocksT(_perm_rows(Wh1) * 0.5)
    wfbT = _as_blocksT(_perm_rows(wfb_full))  # (128, 512)

    b2m = _perm_rows(b2).reshape(4, HID)
    bones = np.zeros((4, G4), f32)
    for g in range(4):
        bones[g, g * HID:(g + 1) * HID] = 1.0

    # per-core transposed inputs: (KX, W*BS), feature on partitions
    xt_cores = []
    for c in range(NCORES):
        xf = xfeat[c * BS:(c + 1) * BS]  # (BS, W, KX)
        xt = np.ascontiguousarray(xf.transpose(2, 1, 0)).reshape(KX, W * BS)
        xt_cores.append(xt.astype(BF16))

    # Pack every weight into one (128, WCOLS) tensor -> single DMA.
    wconst = np.zeros((HID, WCOLS), BF16)
    wconst[:KX, WOFF["wi0"]:WOFF["wi0"] + G4] = wi0T_aug
    wconst[:, WOFF["wh0"]:WOFF["wh0"] + G4] = wh0T
    wconst[:, WOFF["wi1"]:WOFF["wi1"] + G4] = wi1T
    wconst[:, WOFF["wh1"]:WOFF["wh1"] + G4] = wh1T
    wconst[:, WOFF["wfb"]:WOFF["wfb"] + G4] = wfbT
    wconst[:4, WOFF["b2m"]:WOFF["b2m"] + HID] = b2m
    wconst[:4, WOFF["bones"]:WOFF["bones"] + G4] = bones

    return dict(
        xt_cores=xt_cores,
        wconst=wconst,
        weights=dict(
            wi0=wi0T_aug.astype(f32), wh0=wh0T.astype(f32),
            wi1=wi1T.astype(f32), wh1=wh1T.astype(f32),
            wfb=wfbT.astype(f32), b2m=b2m.astype(f32), bones=bones,
        ),
        meanW_h=(0.5 * meanW).astype(f32), mean_b=mean_b,
        norm_std=ns.astype(f32), norm_mean=nm.astype(f32),
    )


def host_post(h2_cores, prep):
    """h2_cores: list of (PRED, HID, BS) arrays of H2=2*h2. -> (B, PRED, 1)."""
    meanW_h = prep["meanW_h"][0]  # (HID,)
    out = np.empty((B, PRED, 1), np.float32)
    for c, h2 in enumerate(h2_cores):
        mn = np.einsum("h,thb->bt", meanW_h, h2.astype(np.float32)) + prep["mean_b"]
        out[c * BS:(c + 1) * BS, :, 0] = mn
    out = out * prep["norm_std"] + prep["norm_mean"]
    return out.astype(np.float32)


_TANH5_MUL = None


def _get_tanh5_mul():
    """Register the fused (sigmoid-gate * poly-tanh) custom DVE op.

    out = in0 * (s0 + t*(s1 + t*imm2)) * in1, t = in0^2.  Registered via the
    documented dve_ops extension point (OPS list + derived tables)."""
    global _TANH5_MUL
    if _TANH5_MUL is not None:
        return _TANH5_MUL
    import concourse.dve_ops as dve_ops
    from concourse.dve_ops import DveOp, get_dve_sub_opcode
    from concourse.dve_spec import Spec, Src0, Src1, C0, C1, C2, sq, lower
    from concourse.dve_uop import DveOpSpec

    name = "TANH5_MUL"
    for existing in dve_ops.OPS:
        if existing.name == name:
            _TANH5_MUL = existing
            return existing
    t = sq(Src0)
    body = (Src0 * (C0 + t * (C1 + t * C2))) * Src1

    def _ref(in0, in1, s0, s1, imm2):
        x = np.asarray(in0, np.float32)
        tt = x * x
        return (x * (s0 + tt * (s1 + tt * imm2))
                * np.asarray(in1, np.float32)).astype(np.float32)

    spec = Spec(body=body, reference=_ref)
    op = DveOp(name, spec, subdim=False, uops_sha={})
    dve_ops.OPS.append(op)
    dve_ops.CUSTOM_DVE_SPECS[name] = spec
    dve_ops._SUB_OPCODE_FOR_NAME[name] = (
        dve_ops._CUSTOM_DVE_ROW_BASE + len(dve_ops.OPS) - 1)
    shas = {}
    for ver in ("v3", "v4"):
        s = DveOpSpec(name=name, opcode=get_dve_sub_opcode(name),
                      uops=lower(spec, ver=ver), rd1_en=True)
        shas[ver] = s.sha(ver)
    object.__setattr__(op, "uops_sha", shas)
    _TANH5_MUL = op
    return op


def build_bass():
    import concourse.bass as bass  # noqa: F401
    import concourse.tile as tile
    from concourse import bacc, mybir

    f32 = mybir.dt.float32
    bf16 = mybir.dt.bfloat16
    AF = mybir.ActivationFunctionType
    ALU = mybir.AluOpType
    OFF = 8  # teacher-phase layer-2 lag (decouples the two recurrence chains)
    tanh5 = _get_tanh5_mul()

    nc = bacc.Bacc("TRN2", target_bir_lowering=False, num_devices=NCORES)
    xt_d = nc.dram_tensor("xt", [KX, W * BS], bf16, kind="ExternalInput")
    wc_d = nc.dram_tensor("wconst", [HID, WCOLS], bf16, kind="ExternalInput")
    h2out_d = nc.dram_tensor("h2out", [PRED, HID, BS], bf16, kind="ExternalOutput")

    with tile.TileContext(nc) as tc:
        with (
            tc.tile_pool(name="const", bufs=1) as const,
            tc.tile_pool(name="xin", bufs=3) as xin,
            tc.tile_pool(name="h1p", bufs=OFF + 3) as h1p,
            tc.tile_pool(name="st", bufs=3) as st,
            tc.tile_pool(name="work", bufs=3) as work,
            tc.tile_pool(name="psA", bufs=3, space="PSUM") as psA,
            tc.tile_pool(name="psB", bufs=3, space="PSUM") as psB,
        ):
            wc = const.tile([HID, WCOLS], bf16, tag="wc", name="wc")
            nc.sync.dma_start(out=wc, in_=wc_d[:, :])
            wt = {
                "wi0": wc[:KX, WOFF["wi0"]:WOFF["wi0"] + G4],
                "wh0": wc[:, WOFF["wh0"]:WOFF["wh0"] + G4],
                "wi1": wc[:, WOFF["wi1"]:WOFF["wi1"] + G4],
                "wh1": wc[:, WOFF["wh1"]:WOFF["wh1"] + G4],
                "wfb": wc[:, WOFF["wfb"]:WOFF["wfb"] + G4],
                "b2m": wc[:4, WOFF["b2m"]:WOFF["b2m"] + HID],
                "bones": wc[:4, WOFF["bones"]:WOFF["bones"] + G4],
            }

            def blk(w, g):
                return w[:, g * HID:(g + 1) * HID]

            # t_ext chains: [C | Tg | Tf | Ti], bf16
            def new_tx(tag):
                t = st.tile([HID, 512], bf16, tag=tag, name=tag)
                return t

            tx1 = new_tx("tx1")
            tx2 = new_tx("tx2")
            nc.vector.memset(tx1[:, 0:128], 0.0)  # C1 = 0
            nc.vector.memset(tx2[:, 0:128], 0.0)  # C2 = 0

            def new_zero(pool, tag, dt):
                t = pool.tile([HID, BS], dt, tag=tag, name=tag)
                nc.vector.memset(t, 0.0)
                return t

            h1 = new_zero(h1p, "h1", bf16)
            h2 = new_zero(st, "h2", bf16)
            h1_hist = {-1: h1}

            # Load the sigmoid table set first (it also contains tanh), so
            # the kernel pays exactly one ACT_TABLE_LOAD.
            sig0 = work.tile([HID, BS], bf16, tag="S1", name="sig0")
            nc.scalar.activation(out=sig0, in_=wc[:, 0:BS], func=AF.Sigmoid)

            # dense back-to-back matmuls: trip the PE HAM activity window so
            # the array doesn't start cold.
            warm = psA.tile([HID, 384], f32, tag="gA", name="warm")
            for k in range(8):
                nc.tensor.matmul(warm, lhsT=wc[:, 0:HID], rhs=wc[:, 0:384],
                                 start=(k == 0), stop=(k == 7))

            xt_sb = None

            def xcol_for(t):
                nonlocal xt_sb
                if t % X_CHUNK == 0:
                    nsteps = min(X_CHUNK, W - t)
                    xt_sb = xin.tile([KX, X_CHUNK * BS], bf16, tag="xt",
                                     name="xt_sb")
                    nc.sync.dma_start(out=xt_sb[:, :nsteps * BS],
                                      in_=xt_d[:, t * BS:(t + nsteps) * BS])
                return xt_sb[:, (t % X_CHUNK) * BS:(t % X_CHUNK + 1) * BS]

            # gate-matmul groups: A = blocks (g,f,i) -> psA tile; B = o -> psB
            def mm_groups(ws_rhs, gA, gB):
                """ws_rhs: list of (weightT_ap, rhs_ap). B (o-gate) mms are
                emitted first so sigma(o) can run on ACT while the A mms
                stream, keeping tanh unblocked right after the A tail."""
                n = len(ws_rhs)
                for k, (wT, rhs) in enumerate(ws_rhs):
                    nc.tensor.matmul(gB, lhsT=blk(wT, 3), rhs=rhs,
                                     start=(k == 0), stop=(k == n - 1))
                for k, (wT, rhs) in enumerate(ws_rhs):
                    for g in range(3):
                        nc.tensor.matmul(blk(gA, g), lhsT=blk(wT, g), rhs=rhs,
                                         start=(k == 0 and g == 0),
                                         stop=(k == n - 1 and g == 2))

            def mm_bias2(gA, gB):
                nc.tensor.matmul(gB, lhsT=wt["b2m"],
                                 rhs=wt["bones"][:, 384:512], start=True,
                                 stop=False)
                nc.tensor.matmul(gA, lhsT=wt["b2m"],
                                 rhs=wt["bones"][:, 0:384], start=True,
                                 stop=False)

            def cell(gA, gB, tx_cur, tx_next, hpool, tag):
                # sigma(o) first: its ACT tick is then covered by the tanh
                # wait that uv already carries, so the custom h-op needs no
                # extra semaphore wait on the scalar engine.
                S = work.tile([HID, BS], bf16, tag=f"S{tag}", name=f"S{tag}")
                nc.scalar.activation(out=S, in_=gB, func=AF.Sigmoid,
                                     scale=2.0)
                nc.scalar.activation(out=tx_cur[:, 128:512], in_=gA,
                                     func=AF.Tanh)
                uv = work.tile([HID, 256], bf16, tag=f"uv{tag}",
                               name=f"uv{tag}")
                nc.vector.scalar_tensor_tensor(
                    out=uv, in0=tx_cur[:, 256:512], scalar=1.0,
                    in1=tx_cur[:, 0:256], op0=ALU.add, op1=ALU.mult)
                nc.vector.scalar_tensor_tensor(
                    out=tx_next[:, 0:128], in0=uv[:, 0:128], scalar=0.5,
                    in1=uv[:, 128:256], op0=ALU.mult, op1=ALU.add)
                h_new = hpool.tile([HID, BS], bf16, tag=f"h{tag}",
                                   name=f"h{tag}")
                nc.vector._custom_dve(tanh5, out=h_new,
                                      in0=tx_next[:, 0:128], in1=S,
                                      s0=P0, s1=P1, imm2=P2)
                return h_new

            # ---------------- teacher phase: L1 stream + L2 stream (lag OFF)
            for i in range(SEQ + OFF):
                j = i - OFF
                if j < 0:
                    # keep the PE HAM window busy until the L2 stream exists
                    wtile = psA.tile([HID, 384], f32, tag="gA", name="wtile")
                    for k in range(6):
                        nc.tensor.matmul(wtile, lhsT=wc[:, 0:HID],
                                         rhs=wc[:, 0:384], start=(k == 0),
                                         stop=(k == 5))
                g2A = g2B = None
                if 0 <= j:
                    g2A = psA.tile([HID, 384], f32, tag="gA", name="g2A")
                    g2B = psB.tile([HID, BS], f32, tag="gB", name="g2B")
                    mm_bias2(g2A, g2B)
                    ws = [(wt["wi1"], h1_hist[j]), (wt["wh1"], h2)]
                    n = len(ws)
                    for k, (wT, rhs) in enumerate(ws):
                        nc.tensor.matmul(g2B, lhsT=blk(wT, 3), rhs=rhs,
                                         start=False, stop=(k == n - 1))
                    for k, (wT, rhs) in enumerate(ws):
                        for g in range(3):
                            nc.tensor.matmul(blk(g2A, g), lhsT=blk(wT, g),
                                             rhs=rhs, start=False,
                                             stop=(k == n - 1 and g == 2))
                g1A = g1B = None
                if i < SEQ:
                    xcol = xcol_for(i)
                    g1A = psA.tile([HID, 384], f32, tag="gA", name="g1A")
                    g1B = psB.tile([HID, BS], f32, tag="gB", name="g1B")
                    mm_groups([(wt["wi0"], xcol), (wt["wh0"], h1_hist[i - 1])],
                              g1A, g1B)
                # L1's cell is emitted first: the static per-step DVE order
                # becomes [uv1, c1, B1, uv2, c2, B2], so the L1 chain's uv
                # never queues behind L2's ops; L2's lag absorbs the wait.
                if g1A is not None:
                    # priority-0 pulls L1's cell ahead of L2's in the static
                    # engine order, so the L1 chain's uv op doesn't queue
                    # behind L2's DVE block; L2's lag absorbs the wait.
                    with tc.high_priority():
                        tx1n = new_tx("tx1")
                        h1_hist[i] = cell(g1A, g1B, tx1, tx1n, h1p, "1")
                        tx1 = tx1n
                    h1_hist.pop(i - OFF - 1, None)
                if g2A is not None:
                    tx2n = new_tx("tx2")
                    h2 = cell(g2A, g2B, tx2, tx2n, st, "2")
                    tx2 = tx2n

            # ---------------- prediction phase: serial, hoisted issue order
            h1 = h1_hist[SEQ - 1]
            for t in range(SEQ, W):
                xcol = xcol_for(t)
                g1A = psA.tile([HID, 384], f32, tag="gA", name="g1A")
                g1B = psB.tile([HID, BS], f32, tag="gB", name="g1B")
                mm_groups([(wt["wi0"], xcol), (wt["wh0"], h1),
                           (wt["wfb"], h2)], g1A, g1B)
                g2A = psA.tile([HID, 384], f32, tag="gA", name="g2A")
                g2B = psB.tile([HID, BS], f32, tag="gB", name="g2B")
                mm_bias2(g2A, g2B)
                nc.tensor.matmul(g2B, lhsT=blk(wt["wh1"], 3), rhs=h2,
                                 start=False, stop=False)
                for g in range(3):
                    nc.tensor.matmul(blk(g2A, g), lhsT=blk(wt["wh1"], g),
                                     rhs=h2, start=False, stop=False)
                tx1n = new_tx("tx1")
                h1 = cell(g1A, g1B, tx1, tx1n, h1p, "1")
                tx1 = tx1n
                nc.tensor.matmul(g2B, lhsT=blk(wt["wi1"], 3), rhs=h1,
                                 start=False, stop=True)
                for g in range(3):
                    nc.tensor.matmul(blk(g2A, g), lhsT=blk(wt["wi1"], g),
                                     rhs=h1, start=False, stop=(g == 2))
                tx2n = new_tx("tx2")
                h2 = cell(g2A, g2B, tx2, tx2n, st, "2")
                tx2 = tx2n
                nc.sync.dma_start(out=h2out_d[t - SEQ], in_=h2)
    nc.compile()
    return nc


_BASS_CACHE = {}


def _get_bass():
    if "nc" not in _BASS_CACHE:
        _BASS_CACHE["nc"] = build_bass()
    return _BASS_CACHE["nc"]


def run(inputs, trace=False):
    """Returns (output, BassKernelResults)."""
    from concourse.bass_utils import run_bass_kernel_spmd

    prep = host_prep(inputs)
    nc = _get_bass()
    in_maps = [{"xt": prep["xt_cores"][c], "wconst": prep["wconst"]}
               for c in range(NCORES)]
    res = run_bass_kernel_spmd(nc, in_maps, core_ids=list(range(NCORES)),
                               trace=trace)
    h2_cores = [r["h2out"] for r in res.results]
    return host_post(h2_cores, prep), res


def kernel(**inputs) -> np.ndarray:
    out, _ = run(inputs, trace=False)
    return out
